# revision 5
# baseline (speedup 1.0000x reference)
"""Trainium2 Bass kernel v3 for CRF Viterbi decode (nn_CRFLayer).

The axon/PJRT execution path costs ~33-45us PER INSTRUCTION regardless of
data size (measured), so this kernel minimizes total instruction count:

Forward (exact serial recursion, bit-identical to the jax reference):
  3 plain DVE instructions per step on [64p seqs]:
    cand[p,j,i] = s[p,i] + TrT[j,i]          tensor_tensor  (2304 elems)
    pre[p,j]    = max_i cand                  grouped tensor_reduce
    s'[p,j]     = pre + em_t[p,j]             tensor_tensor  (48 elems, in place
                                              in the hist block tile)
  hist (= s_t for every t) streams to DRAM in TB-step blocks.

Final argmax at t=S-1 with first-occurrence semantics via the min-encode:
  tag_enc = min_j( (fin[j]==max) ? j-BIG : 0 ) = argmax - BIG.
Then hist row S-1 in DRAM is REPLACED by LARGE*onehot(tag), which forces any
backtrace chase passing t=S-1 to land on the true final tag.

Backtrace: C time-chunks chased in lockstep (one instruction processes all
chunks), each warmed up DELTA steps from a dummy seed above its range --
backpointer maps coalesce in ~32 steps so the chunk entry tag is exact with
overwhelming probability; chunk boundaries are verified exactly on device
(specfail output) and kernel() falls back to a C=1 exact serial chase on any
mismatch. 8 plain DVE instructions per lockstep round:
    oh   = (iotam == cur)                 one-hot of current tags  [p,C,48]
    tp   = oh * Tr  (broadcast [c,i,j])   [p,C,48,48]
    tcol = sum_j tp                       = Tr[:, tag] per chunk   [p,C,48]
    c48  = hist_t + tcol
    m    = max_i c48
    sel  = (c48 == m)
    val  = sel * iotam
    new  = min_i val  -> written straight into the paths buffer (strided view)
Tags are carried min-ENCODED as tag-BIG throughout; +BIG is applied once at
the end.
"""

import sys
from contextlib import ExitStack

import numpy as np

sys.path.insert(0, "/opt/trn_rl_repo")

import concourse.bass as bass  # noqa: E402
import concourse.tile as tile  # noqa: E402
from concourse import bacc, mybir  # noqa: E402

F32 = mybir.dt.float32
I32 = mybir.dt.int32

NUM_TAGS = 48
BATCH = 512
SEQ_LEN = 1024
N_CORES = 8
B_LOC = BATCH // N_CORES  # 64 sequences per core
BIG = 1024.0
LARGE = 1.0e6

add = mybir.AluOpType.add
vmax = mybir.AluOpType.max
vmin = mybir.AluOpType.min
mult = mybir.AluOpType.mult
iseq = mybir.AluOpType.is_equal
neq = mybir.AluOpType.not_equal
X = mybir.AxisListType.X


def build_nc(
    S: int = SEQ_LEN,
    B: int = B_LOC,
    T: int = NUM_TAGS,
    TB: int = 128,     # forward block (em in / hist out) in steps
    C: int = 16,       # backtrace chunks (1 = exact serial chase)
    DELTA: int = 24,   # speculative warmup steps per chunk
    W: int = 8,        # backtrace hist staging window (rounds per DMA block)
    CH: int = 8,       # chunks per gather half-pass (SBUF: CH*9.2KB tp tile)
    reps: int = 1,
    fwd_only: bool = False,
):
    assert S % TB == 0
    nblk = S // TB
    L = S // C
    assert (L + DELTA) % W == 0 or C == 1
    SP = (C + 1) * L  # hist rows incl. DELTA virtual rows (DELTA < L required)
    assert DELTA < L

    nc = bacc.Bacc("TRN2", target_bir_lowering=False, debug=False, num_devices=N_CORES)

    em_d = nc.dram_tensor("emissions", [B, S, T], F32, kind="ExternalInput")
    trans_d = nc.dram_tensor("transitions", [T, T], F32, kind="ExternalInput")
    start_d = nc.dram_tensor("start_transitions", [T], F32, kind="ExternalInput")
    end_d = nc.dram_tensor("end_transitions", [T], F32, kind="ExternalInput")
    paths_d = nc.dram_tensor("paths", [B, S], I32, kind="ExternalOutput")
    specfail_d = nc.dram_tensor("specfail", [B, 16], F32, kind="ExternalOutput")
    hist_d = nc.dram_tensor("hist", [B, SP, T], F32, kind="Internal")

    with tile.TileContext(nc) as tc, ExitStack() as ctx:
        const = ctx.enter_context(tc.tile_pool(name="const", bufs=1))

        # ---- constants -------------------------------------------------
        t_ap = trans_d.ap()  # [i, j]
        # trT_rep[p, j, i] = Tr[i, j] ; tr_rep[p, i, j] = Tr[i, j]
        trT_flat = const.tile([1, T * T], F32)
        nc.sync.dma_start(
            trT_flat[:].rearrange("p (j i) -> p j i", j=T), t_ap.transpose([1, 0]).unsqueeze(0)
        )
        tr_flat = const.tile([1, T * T], F32)
        nc.sync.dma_start(
            tr_flat[:].rearrange("p (i j) -> p i j", i=T), t_ap.unsqueeze(0)
        )
        trT_rep = const.tile([B, T * T], F32)
        nc.gpsimd.partition_broadcast(trT_rep[:], trT_flat[:])
        tr_rep = const.tile([B, T * T], F32)
        nc.gpsimd.partition_broadcast(tr_rep[:], tr_flat[:])

        start_b = const.tile([B, T], F32)
        nc.sync.dma_start(start_b[:], start_d.ap().unsqueeze(0).broadcast_to([B, T]))
        end_b = const.tile([B, T], F32)
        nc.sync.dma_start(end_b[:], end_d.ap().unsqueeze(0).broadcast_to([B, T]))

        # iotam[p, j] = j - BIG
        iota_i = const.tile([B, T], I32)
        nc.gpsimd.iota(iota_i[:], pattern=[[1, T]], base=0, channel_multiplier=0)
        iotam = const.tile([B, T], F32)
        nc.vector.tensor_scalar(iotam[:], iota_i[:], -BIG, None, op0=add)

        # paths buffer, min-encoded; [B, (C+2)*L] flat, 3D views via rearrange
        pe = const.tile([B, (C + 2) * L], F32)
        bnd = const.tile([B, max(C - 1, 1)], F32)
        sfail = const.tile([B, 16], F32)
        fin = const.tile([B, T], F32)
        mfin = const.tile([B, 1], F32)
        selS = const.tile([B, T], F32)
        valS = const.tile([B, T], F32)
        tagS1 = const.tile([B, 1], F32)
        ohL = const.tile([B, T], F32)
        pathsf = const.tile([B, S], F32)
        paths_i = const.tile([B, S], I32)

        nc.vector.memset(sfail[:], 0.0)
        if DELTA > 0:
            zpad = const.tile([B, DELTA, T], F32)
            nc.vector.memset(zpad[:], 0.0)
            nc.sync.dma_start(hist_d.ap()[:, S : S + DELTA, :], zpad[:])

        def pe3(c0, c1, o):
            return pe[:].rearrange("p (c l) -> p c l", l=L)[:, c0:c1, o : o + 1]

        for _rep in range(reps):
            nc.vector.memset(pe[:], 0.0 - BIG)  # chase seeds (tag 0) everywhere

            # =================== forward ===================================
            with ExitStack() as fctx:
                emp = fctx.enter_context(tc.tile_pool(name="emp", bufs=2))
                sfp = fctx.enter_context(tc.tile_pool(name="sfp", bufs=2))
                wrk = fctx.enter_context(tc.tile_pool(name="fwrk", bufs=2))

                sf_prev = None
                for blk in range(nblk):
                    em_t = emp.tile([B, TB, T], F32, tag="em")
                    nc.sync.dma_start(em_t[:], em_d.ap()[:, blk * TB : (blk + 1) * TB, :])
                    sf = sfp.tile([B, TB, T], F32, tag="sf")
                    for off in range(TB):
                        t = blk * TB + off
                        if t == 0:
                            nc.vector.tensor_add(sf[:, 0, :], start_b[:], em_t[:, 0, :])
                        else:
                            sprev = sf[:, off - 1, :] if off > 0 else sf_prev[:, TB - 1, :]
                            cand = wrk.tile([B, T, T], F32, tag="cand")
                            nc.vector.tensor_tensor(
                                cand[:],
                                sprev.unsqueeze(1).broadcast_to([B, T, T]),
                                trT_rep[:].rearrange("p (j i) -> p j i", j=T),
                                op=add,
                            )
                            nc.vector.tensor_reduce(sf[:, off, :], cand[:], axis=X, op=vmax)
                            nc.vector.tensor_tensor(
                                sf[:, off, :], sf[:, off, :], em_t[:, off, :], op=add
                            )
                    nc.sync.dma_start(
                        hist_d.ap()[:, blk * TB : (blk + 1) * TB, :], sf[:]
                    )
                    sf_prev = sf

                # ---- final argmax at t = S-1 (first-occurrence, min-encoded)
                nc.vector.tensor_add(fin[:], sf_prev[:, TB - 1, :], end_b[:])
                nc.vector.tensor_reduce(mfin[:], fin[:], axis=X, op=vmax)
                nc.vector.tensor_tensor(
                    selS[:], fin[:], mfin[:].broadcast_to([B, T]), op=iseq
                )
                nc.vector.tensor_tensor(valS[:], selS[:], iotam[:], op=mult)
                nc.vector.tensor_reduce(tagS1[:], valS[:], axis=X, op=vmin)
                # forced one-hot row at S-1: LARGE * (iotam == tag_enc)
                nc.vector.tensor_tensor(
                    ohL[:], iotam[:], tagS1[:].broadcast_to([B, T]), op=iseq
                )
                nc.vector.tensor_scalar(ohL[:], ohL[:], LARGE, None, op0=mult)
                nc.sync.dma_start(hist_d.ap()[:, S - 1 : S, :], ohL[:].unsqueeze(1))

            if fwd_only:
                nc.vector.tensor_copy(paths_i[:], pe[:, 0:S])
                nc.sync.dma_start(paths_d.ap()[:], paths_i[:])
                nc.sync.dma_start(specfail_d.ap()[:], sfail[:])
                continue

            # =================== backtrace =================================
            with ExitStack() as bctx:
                hip = bctx.enter_context(tc.tile_pool(name="hip", bufs=2))
                bwrk = bctx.enter_context(tc.tile_pool(name="bwrk", bufs=2))
                tpp = bctx.enter_context(tc.tile_pool(name="tpp", bufs=1))

                hist4 = hist_d.ap().rearrange("b (c l) t -> b c l t", l=L)
                nrounds = L + DELTA
                nwin = nrounds // W
                hb = {}

                def load_window(wi):
                    # window wi covers rounds [wi*W, (wi+1)*W); chunk c reads
                    # rows t = (c+1)*L - 1 + DELTA - r; shared in-block offset
                    # off = W-1-(r - wi*W); block base rows (c+1)*L + q,
                    # q = DELTA - wi*W - W
                    if wi >= nwin:
                        return
                    q = DELTA - wi * W - W
                    hbt = hip.tile([B, C, W, T], F32, tag="hb")
                    if q >= 0:
                        src = hist4[:, 1 : C + 1, q : q + W, :]
                    else:
                        src = hist4[:, 0:C, L + q : L + q + W, :]
                    nc.sync.dma_start(hbt[:], src)
                    hb[wi] = hbt

                load_window(0)
                load_window(1)

                for r in range(nrounds):
                    wi, roff = r // W, r % W
                    if roff == 0 and r > 0:
                        load_window(wi + 1)
                    off = W - 1 - roff
                    base_prev = L + DELTA - r
                    base_cur = base_prev - 1
                    c0p, op_ = base_prev // L, base_prev % L
                    c0c, oc_ = base_cur // L, base_cur % L

                    if r == nrounds - 1 and C > 1:
                        # capture speculative boundary tags before the final
                        # round overwrites positions (c+1)*L
                        nc.vector.tensor_copy(bnd[:].unsqueeze(2), pe3(1, C, 0))

                    cur = pe3(c0p, c0p + C, op_)  # [B, C, 1] previous tags
                    oh = bwrk.tile([B, C, T], F32, tag="oh")
                    nc.vector.tensor_tensor(
                        oh[:],
                        iotam[:].unsqueeze(1).broadcast_to([B, C, T]),
                        cur.broadcast_to([B, C, T]),
                        op=iseq,
                    )
                    tcol = bwrk.tile([B, C, T], F32, tag="tcol")
                    for h0 in range(0, C, CH):
                        h1 = min(h0 + CH, C)
                        hw = h1 - h0
                        tp = tpp.tile([B, CH, T, T], F32, tag="tp")
                        nc.vector.tensor_tensor(
                            tp[:, 0:hw],
                            oh[:, h0:h1].unsqueeze(2).broadcast_to([B, hw, T, T]),
                            tr_rep[:].rearrange("p (i j) -> p i j", i=T)
                            .unsqueeze(1)
                            .broadcast_to([B, hw, T, T]),
                            op=mult,
                        )
                        nc.vector.tensor_reduce(
                            tcol[:, h0:h1],
                            tp[:, 0:hw].rearrange("p c i j -> p (c i) j"),
                            axis=X,
                            op=add,
                        )
                    c48 = bwrk.tile([B, C, T], F32, tag="c48")
                    nc.vector.tensor_tensor(c48[:], hb[wi][:, :, off, :], tcol[:], op=add)
                    m = bwrk.tile([B, C, 1], F32, tag="m")
                    nc.vector.tensor_reduce(m[:], c48[:], axis=X, op=vmax)
                    sel = bwrk.tile([B, C, T], F32, tag="sel")
                    nc.vector.tensor_tensor(
                        sel[:], c48[:], m[:].broadcast_to([B, C, T]), op=iseq
                    )
                    val = bwrk.tile([B, C, T], F32, tag="val")
                    nc.vector.tensor_tensor(
                        val[:], sel[:], iotam[:].unsqueeze(1).broadcast_to([B, C, T]), op=mult
                    )
                    nc.vector.tensor_reduce(
                        pe3(c0c, c0c + C, oc_), val[:], axis=X, op=vmin
                    )

                if C > 1:
                    nc.vector.tensor_tensor(
                        sfail[:, 0 : C - 1].unsqueeze(2),
                        bnd[:].unsqueeze(2),
                        pe3(1, C, 0),
                        op=neq,
                    )

            # ---- emit outputs -----------------------------------------
            nc.vector.tensor_scalar(pathsf[:], pe[:, 0:S], BIG, None, op0=add)
            nc.vector.tensor_copy(paths_i[:], pathsf[:])
            nc.sync.dma_start(paths_d.ap()[:], paths_i[:])
            nc.sync.dma_start(specfail_d.ap()[:], sfail[:])

    nc.compile()
    return nc


def _run(nc, emissions, transitions, start_transitions, end_transitions):
    from concourse.bass_utils import run_bass_kernel_spmd

    in_maps = []
    for c in range(N_CORES):
        in_maps.append(
            {
                "emissions": np.ascontiguousarray(
                    emissions[c * B_LOC : (c + 1) * B_LOC], dtype=np.float32
                ),
                "transitions": transitions,
                "start_transitions": start_transitions,
                "end_transitions": end_transitions,
            }
        )
    return run_bass_kernel_spmd(nc, in_maps, list(range(N_CORES)))


def kernel(emissions, mask, transitions, start_transitions, end_transitions):
    """Full-input entry point: shards batch over 8 cores, runs SPMD, gathers."""
    emissions = np.ascontiguousarray(np.asarray(emissions), dtype=np.float32)
    transitions = np.ascontiguousarray(np.asarray(transitions), dtype=np.float32)
    start_transitions = np.ascontiguousarray(
        np.asarray(start_transitions), dtype=np.float32
    )
    end_transitions = np.ascontiguousarray(np.asarray(end_transitions), dtype=np.float32)

    nc = build_nc()
    res = _run(nc, emissions, transitions, start_transitions, end_transitions)
    fail = any(np.asarray(r["specfail"]).any() for r in res.results)
    if fail:
        # speculation miss (astronomically rare): exact serial chase
        nc2 = build_nc(C=1, DELTA=0, W=16)
        res = _run(nc2, emissions, transitions, start_transitions, end_transitions)
    out = np.concatenate([np.asarray(r["paths"]) for r in res.results], axis=0)
    return out.astype(np.int32)


# revision 7
# speedup vs baseline: 1.1538x; 1.1538x over previous
"""Trainium2 Bass kernel v3 for CRF Viterbi decode (nn_CRFLayer).

The axon/PJRT execution path costs ~33-45us PER INSTRUCTION regardless of
data size (measured), so this kernel minimizes total instruction count:

Forward (exact serial recursion, bit-identical to the jax reference):
  3 plain DVE instructions per step on [64p seqs]:
    cand[p,j,i] = s[p,i] + TrT[j,i]          tensor_tensor  (2304 elems)
    pre[p,j]    = max_i cand                  grouped tensor_reduce
    s'[p,j]     = pre + em_t[p,j]             tensor_tensor  (48 elems, in place
                                              in the hist block tile)
  hist (= s_t for every t) streams to DRAM in TB-step blocks.

Final argmax at t=S-1 with first-occurrence semantics via the min-encode:
  tag_enc = min_j( (fin[j]==max) ? j-BIG : 0 ) = argmax - BIG.
Then hist row S-1 in DRAM is REPLACED by LARGE*onehot(tag), which forces any
backtrace chase passing t=S-1 to land on the true final tag.

Backtrace: C time-chunks chased in lockstep (one instruction processes all
chunks), each warmed up DELTA steps from a dummy seed above its range --
backpointer maps coalesce in ~32 steps so the chunk entry tag is exact with
overwhelming probability; chunk boundaries are verified exactly on device
(specfail output) and kernel() falls back to a C=1 exact serial chase on any
mismatch. 8 plain DVE instructions per lockstep round:
    oh   = (iotam == cur)                 one-hot of current tags  [p,C,48]
    tp   = oh * Tr  (broadcast [c,i,j])   [p,C,48,48]
    tcol = sum_j tp                       = Tr[:, tag] per chunk   [p,C,48]
    c48  = hist_t + tcol
    m    = max_i c48
    sel  = (c48 == m)
    val  = sel * iotam
    new  = min_i val  -> written straight into the paths buffer (strided view)
Tags are carried min-ENCODED as tag-BIG throughout; +BIG is applied once at
the end.
"""

import sys
from contextlib import ExitStack

import numpy as np

sys.path.insert(0, "/opt/trn_rl_repo")

import concourse.bass as bass  # noqa: E402
import concourse.tile as tile  # noqa: E402
from concourse import bacc, mybir  # noqa: E402

F32 = mybir.dt.float32
I32 = mybir.dt.int32

NUM_TAGS = 48
BATCH = 512
SEQ_LEN = 1024
N_CORES = 4  # 4x128 dispatches ~19% faster than 8x64 (same instruction count)
B_LOC = BATCH // N_CORES  # 128 sequences per core (one per partition)
BIG = 1024.0
LARGE = 1.0e6

add = mybir.AluOpType.add
vmax = mybir.AluOpType.max
vmin = mybir.AluOpType.min
mult = mybir.AluOpType.mult
iseq = mybir.AluOpType.is_equal
neq = mybir.AluOpType.not_equal
X = mybir.AxisListType.X


def build_nc(
    S: int = SEQ_LEN,
    B: int = B_LOC,
    T: int = NUM_TAGS,
    TB: int = 128,     # forward block (em in / hist out) in steps
    C: int = 16,       # backtrace chunks (1 = exact serial chase)
    DELTA: int = 24,   # speculative warmup steps per chunk
    W: int = 8,        # backtrace hist staging window (rounds per DMA block)
    CH: int = 8,       # chunks per gather half-pass (SBUF: CH*9.2KB tp tile)
    reps: int = 1,
    fwd_only: bool = False,
    ncores: int = N_CORES,
):
    assert S % TB == 0
    nblk = S // TB
    L = S // C
    assert (L + DELTA) % W == 0 or C == 1
    SP = (C + 1) * L  # hist rows incl. DELTA virtual rows (DELTA < L required)
    assert DELTA < L

    nc = bacc.Bacc("TRN2", target_bir_lowering=False, debug=False, num_devices=ncores)

    em_d = nc.dram_tensor("emissions", [B, S, T], F32, kind="ExternalInput")
    trans_d = nc.dram_tensor("transitions", [T, T], F32, kind="ExternalInput")
    start_d = nc.dram_tensor("start_transitions", [T], F32, kind="ExternalInput")
    end_d = nc.dram_tensor("end_transitions", [T], F32, kind="ExternalInput")
    paths_d = nc.dram_tensor("paths", [B, S], I32, kind="ExternalOutput")
    specfail_d = nc.dram_tensor("specfail", [B, 16], F32, kind="ExternalOutput")
    hist_d = nc.dram_tensor("hist", [B, SP, T], F32, kind="Internal")

    with tile.TileContext(nc) as tc, ExitStack() as ctx:
        const = ctx.enter_context(tc.tile_pool(name="const", bufs=1))

        # ---- constants -------------------------------------------------
        t_ap = trans_d.ap()  # [i, j]
        # trT_rep[p, j, i] = Tr[i, j] ; tr_rep[p, i, j] = Tr[i, j]
        trT_flat = const.tile([1, T * T], F32)
        nc.sync.dma_start(
            trT_flat[:].rearrange("p (j i) -> p j i", j=T), t_ap.transpose([1, 0]).unsqueeze(0)
        )
        tr_flat = const.tile([1, T * T], F32)
        nc.sync.dma_start(
            tr_flat[:].rearrange("p (i j) -> p i j", i=T), t_ap.unsqueeze(0)
        )
        trT_rep = const.tile([B, T * T], F32)
        nc.gpsimd.partition_broadcast(trT_rep[:], trT_flat[:])
        tr_rep = const.tile([B, T * T], F32)
        nc.gpsimd.partition_broadcast(tr_rep[:], tr_flat[:])

        start_b = const.tile([B, T], F32)
        nc.sync.dma_start(start_b[:], start_d.ap().unsqueeze(0).broadcast_to([B, T]))
        end_b = const.tile([B, T], F32)
        nc.sync.dma_start(end_b[:], end_d.ap().unsqueeze(0).broadcast_to([B, T]))

        # iotam[p, j] = j - BIG
        iota_i = const.tile([B, T], I32)
        nc.gpsimd.iota(iota_i[:], pattern=[[1, T]], base=0, channel_multiplier=0)
        iotam = const.tile([B, T], F32)
        nc.vector.tensor_scalar(iotam[:], iota_i[:], -BIG, None, op0=add)

        # paths buffer, min-encoded; [B, (C+2)*L] flat, 3D views via rearrange
        pe = const.tile([B, (C + 2) * L], F32)
        bnd = const.tile([B, max(C - 1, 1)], F32)
        sfail = const.tile([B, 16], F32)
        fin = const.tile([B, T], F32)
        mfin = const.tile([B, 1], F32)
        selS = const.tile([B, T], F32)
        valS = const.tile([B, T], F32)
        tagS1 = const.tile([B, 1], F32)
        ohL = const.tile([B, T], F32)
        pathsf = const.tile([B, S], F32)
        paths_i = const.tile([B, S], I32)

        nc.vector.memset(sfail[:], 0.0)
        if DELTA > 0:
            zpad = const.tile([B, DELTA, T], F32)
            nc.vector.memset(zpad[:], 0.0)
            nc.sync.dma_start(hist_d.ap()[:, S : S + DELTA, :], zpad[:])

        def pe3(c0, c1, o):
            return pe[:].rearrange("p (c l) -> p c l", l=L)[:, c0:c1, o : o + 1]

        for _rep in range(reps):
            nc.vector.memset(pe[:], 0.0 - BIG)  # chase seeds (tag 0) everywhere

            # =================== forward ===================================
            with ExitStack() as fctx:
                emp = fctx.enter_context(tc.tile_pool(name="emp", bufs=2))
                sfp = fctx.enter_context(tc.tile_pool(name="sfp", bufs=2))
                wrk = fctx.enter_context(tc.tile_pool(name="fwrk", bufs=2))

                sf_prev = None
                for blk in range(nblk):
                    em_t = emp.tile([B, TB, T], F32, tag="em")
                    nc.sync.dma_start(em_t[:], em_d.ap()[:, blk * TB : (blk + 1) * TB, :])
                    sf = sfp.tile([B, TB, T], F32, tag="sf")
                    for off in range(TB):
                        t = blk * TB + off
                        if t == 0:
                            nc.vector.tensor_add(sf[:, 0, :], start_b[:], em_t[:, 0, :])
                        else:
                            sprev = sf[:, off - 1, :] if off > 0 else sf_prev[:, TB - 1, :]
                            cand = wrk.tile([B, T, T], F32, tag="cand")
                            nc.vector.tensor_tensor(
                                cand[:],
                                sprev.unsqueeze(1).broadcast_to([B, T, T]),
                                trT_rep[:].rearrange("p (j i) -> p j i", j=T),
                                op=add,
                            )
                            nc.vector.tensor_reduce(sf[:, off, :], cand[:], axis=X, op=vmax)
                            nc.vector.tensor_tensor(
                                sf[:, off, :], sf[:, off, :], em_t[:, off, :], op=add
                            )
                    nc.sync.dma_start(
                        hist_d.ap()[:, blk * TB : (blk + 1) * TB, :], sf[:]
                    )
                    sf_prev = sf

                # ---- final argmax at t = S-1 (first-occurrence, min-encoded)
                nc.vector.tensor_add(fin[:], sf_prev[:, TB - 1, :], end_b[:])
                nc.vector.tensor_reduce(mfin[:], fin[:], axis=X, op=vmax)
                nc.vector.tensor_tensor(
                    selS[:], fin[:], mfin[:].broadcast_to([B, T]), op=iseq
                )
                nc.vector.tensor_tensor(valS[:], selS[:], iotam[:], op=mult)
                nc.vector.tensor_reduce(tagS1[:], valS[:], axis=X, op=vmin)
                # forced one-hot row at S-1: LARGE * (iotam == tag_enc)
                nc.vector.tensor_tensor(
                    ohL[:], iotam[:], tagS1[:].broadcast_to([B, T]), op=iseq
                )
                nc.vector.tensor_scalar(ohL[:], ohL[:], LARGE, None, op0=mult)
                nc.sync.dma_start(hist_d.ap()[:, S - 1 : S, :], ohL[:].unsqueeze(1))

            if fwd_only:
                nc.vector.tensor_copy(paths_i[:], pe[:, 0:S])
                nc.sync.dma_start(paths_d.ap()[:], paths_i[:])
                nc.sync.dma_start(specfail_d.ap()[:], sfail[:])
                continue

            # =================== backtrace =================================
            with ExitStack() as bctx:
                hip = bctx.enter_context(tc.tile_pool(name="hip", bufs=2))
                bwrk = bctx.enter_context(tc.tile_pool(name="bwrk", bufs=2))
                tpp = bctx.enter_context(tc.tile_pool(name="tpp", bufs=1))

                hist4 = hist_d.ap().rearrange("b (c l) t -> b c l t", l=L)
                nrounds = L + DELTA
                nwin = nrounds // W
                hb = {}

                def load_window(wi):
                    # window wi covers rounds [wi*W, (wi+1)*W); chunk c reads
                    # rows t = (c+1)*L - 1 + DELTA - r; shared in-block offset
                    # off = W-1-(r - wi*W); block base rows (c+1)*L + q,
                    # q = DELTA - wi*W - W
                    if wi >= nwin:
                        return
                    q = DELTA - wi * W - W
                    hbt = hip.tile([B, C, W, T], F32, tag="hb")
                    if q >= 0:
                        src = hist4[:, 1 : C + 1, q : q + W, :]
                    else:
                        src = hist4[:, 0:C, L + q : L + q + W, :]
                    nc.sync.dma_start(hbt[:], src)
                    hb[wi] = hbt

                load_window(0)
                load_window(1)

                for r in range(nrounds):
                    wi, roff = r // W, r % W
                    if roff == 0 and r > 0:
                        load_window(wi + 1)
                    off = W - 1 - roff
                    base_prev = L + DELTA - r
                    base_cur = base_prev - 1
                    c0p, op_ = base_prev // L, base_prev % L
                    c0c, oc_ = base_cur // L, base_cur % L

                    if r == nrounds - 1 and C > 1:
                        # capture speculative boundary tags before the final
                        # round overwrites positions (c+1)*L
                        nc.vector.tensor_copy(bnd[:].unsqueeze(2), pe3(1, C, 0))

                    cur = pe3(c0p, c0p + C, op_)  # [B, C, 1] previous tags
                    oh = bwrk.tile([B, C, T], F32, tag="oh")
                    nc.vector.tensor_tensor(
                        oh[:],
                        iotam[:].unsqueeze(1).broadcast_to([B, C, T]),
                        cur.broadcast_to([B, C, T]),
                        op=iseq,
                    )
                    tcol = bwrk.tile([B, C, T], F32, tag="tcol")
                    for h0 in range(0, C, CH):
                        h1 = min(h0 + CH, C)
                        hw = h1 - h0
                        tp = tpp.tile([B, CH, T, T], F32, tag="tp")
                        nc.vector.tensor_tensor(
                            tp[:, 0:hw],
                            oh[:, h0:h1].unsqueeze(2).broadcast_to([B, hw, T, T]),
                            tr_rep[:].rearrange("p (i j) -> p i j", i=T)
                            .unsqueeze(1)
                            .broadcast_to([B, hw, T, T]),
                            op=mult,
                        )
                        nc.vector.tensor_reduce(
                            tcol[:, h0:h1],
                            tp[:, 0:hw].rearrange("p c i j -> p (c i) j"),
                            axis=X,
                            op=add,
                        )
                    c48 = bwrk.tile([B, C, T], F32, tag="c48")
                    nc.vector.tensor_tensor(c48[:], hb[wi][:, :, off, :], tcol[:], op=add)
                    m = bwrk.tile([B, C, 1], F32, tag="m")
                    nc.vector.tensor_reduce(m[:], c48[:], axis=X, op=vmax)
                    sel = bwrk.tile([B, C, T], F32, tag="sel")
                    nc.vector.tensor_tensor(
                        sel[:], c48[:], m[:].broadcast_to([B, C, T]), op=iseq
                    )
                    val = bwrk.tile([B, C, T], F32, tag="val")
                    nc.vector.tensor_tensor(
                        val[:], sel[:], iotam[:].unsqueeze(1).broadcast_to([B, C, T]), op=mult
                    )
                    nc.vector.tensor_reduce(
                        pe3(c0c, c0c + C, oc_), val[:], axis=X, op=vmin
                    )

                if C > 1:
                    nc.vector.tensor_tensor(
                        sfail[:, 0 : C - 1].unsqueeze(2),
                        bnd[:].unsqueeze(2),
                        pe3(1, C, 0),
                        op=neq,
                    )

            # ---- emit outputs -----------------------------------------
            nc.vector.tensor_scalar(pathsf[:], pe[:, 0:S], BIG, None, op0=add)
            nc.vector.tensor_copy(paths_i[:], pathsf[:])
            nc.sync.dma_start(paths_d.ap()[:], paths_i[:])
            nc.sync.dma_start(specfail_d.ap()[:], sfail[:])

    nc.compile()
    return nc


def _run(nc, emissions, transitions, start_transitions, end_transitions):
    from concourse.bass_utils import run_bass_kernel_spmd

    in_maps = []
    for c in range(N_CORES):
        in_maps.append(
            {
                "emissions": np.ascontiguousarray(
                    emissions[c * B_LOC : (c + 1) * B_LOC], dtype=np.float32
                ),
                "transitions": transitions,
                "start_transitions": start_transitions,
                "end_transitions": end_transitions,
            }
        )
    return run_bass_kernel_spmd(nc, in_maps, list(range(N_CORES)))


def kernel(emissions, mask, transitions, start_transitions, end_transitions):
    """Full-input entry point: shards batch over 8 cores, runs SPMD, gathers."""
    emissions = np.ascontiguousarray(np.asarray(emissions), dtype=np.float32)
    transitions = np.ascontiguousarray(np.asarray(transitions), dtype=np.float32)
    start_transitions = np.ascontiguousarray(
        np.asarray(start_transitions), dtype=np.float32
    )
    end_transitions = np.ascontiguousarray(np.asarray(end_transitions), dtype=np.float32)

    nc = build_nc()
    res = _run(nc, emissions, transitions, start_transitions, end_transitions)
    fail = any(np.asarray(r["specfail"]).any() for r in res.results)
    if fail:
        # speculation miss (astronomically rare): exact serial chase
        nc2 = build_nc(C=1, DELTA=0, W=16)
        res = _run(nc2, emissions, transitions, start_transitions, end_transitions)
    out = np.concatenate([np.asarray(r["paths"]) for r in res.results], axis=0)
    return out.astype(np.int32)
